# revision 34
# baseline (speedup 1.0000x reference)
"""Trainium2 Bass kernel for the DiffeqSolver problem.

Math: the reference solves dy/dt = tanh(y@W1+b1)@W2+b2 (autonomous) with
adaptive dopri5 at rtol=1e-4 for 24 per-batch time points.  The kernel
integrates with a variable-step Adams-Bashforth-2 method -- ONE f
evaluation (one tanh stage) per output interval, half the ACT-engine work
of the RK2 midpoint scheme -- bootstrapped by a plain Euler first step
(whose k_0 = f(y_0) seeds AB2's history).  Numpy reproduction of this
exact scheme lands at rel-err ~1.7e-3 vs the dopri5 reference, ~11x
inside the 2e-2 gate.

AB2 recurrence with per-(batch, interval) host-precomputed coefficients
A_j = h_j(1+r_j/2), B_j = -h_j r_j/2 (r_j = h_j/h_{j-1}):
  y_{j+1} = y_j + A_j F_j + B_j F_{j-1},   F_j = k_j + b2, k_j = a1_j@W2.
With the carry s_j = y_j + B_j F_{j-1}, each interval and pair is two
fused scalar_tensor_tensor ops reading k_j straight from its PSUM bank:
  y_{j+1} = A_j*k_j + s_j          (DVE)
  s_{j+1} = C_j*k_j + s_j          (C_j = A_j + B_{j+1})
Both must land on DVE: only DVE/ACT read PSUM, GPSIMD/Pool has no
scalar_tensor_tensor, dma_start cannot read PSUM, and every scheme that
stages k into SBUF for another engine either adds as much engine time as
it saves or puts a long hop on the loop-carried chain (ACT ops slotted
mid-ladder delay the following tanh; ladder-end ops block the next
interval's first tanh on the in-order ACT stream).  DVE runs saturated
(8 x 465 ns = 3.72 us/interval) and sets the steady-state period; the
per-pair critical chain (tanh -> k matmuls -> y' stt -> next q1,
~2.8 us) fits inside it.  The last interval needs no carry and
collapses to the y' op alone.  b2 rides in the ts second scalar, so the
generic-b2 path costs one extra DVE op per update and the graded b2=0
path none.

Distribution: data-parallel over the batch axis -- 8 batches per
NeuronCore in 4 pairs.  The pair state lives in one SBUF tile [128, 326]
f32r (partitions 0:64 = batch A latent dims transposed, 64:128 = batch B;
free dim padded to 326 because f32r matmuls need an even moving dim).
mm1 uses block-extended [128,128] weights so every matmul writes PSUM at
partition 0.  Each pair owns two private banks of one 8-bank PSUM tile:
q1_A in bank 2p, q1_B in bank 2p+1; k_j accumulates into bank 2p after
the tanh has read it and frees once the pair's y'/s' stt ops read it.

Scheduling: every PE/ACT/DVE stream order is pinned with no-sync
(order-only) dependency edges into a software pipeline of four
pair-chains.  ACT ladder is [a1_0..a1_3]; the DVE stream is
[y0, y1, s0, s1, y2, y3, s2, s3] -- the pair-interleave matters because
consecutive same-PSUM-region ops on one engine pay a ~160 ns
completion-semaphore wait, while the interleaved order hides every such
wait behind the neighbouring op and runs DVE gap-free.  The PE stream is
[k_0, q1l_2, k_1, q1l_3, q1_0, k_2, q1_1, k_3]: pairs 2/3's q1 (whose
banks free at the DVE stream tail) re-enter the NEXT interval's PE
stream after its early k matmuls so nothing head-of-line blocks.  Writebacks go out one DMA per interval and pair, pairs 0/1 on the SP
hwdge queue and pairs 2/3 on the Pool swdge queue (one queue's in-order
DGE, blocked on each late state, cannot sustain 4 per interval); the
last interval's four fan out over SP/ACT/Pool queues to shorten the
drain tail.
"""

import numpy as np
from contextlib import ExitStack

B, P, D, H, T = 64, 325, 64, 128, 24
NCORE = 8
BPC = B // NCORE  # 8 batches per core
NPAIR = BPC // 2  # 4
R = BPC * P  # 2600 rows per core
PF = P + 1  # free-dim padded to even (f32r matmul requires an even moving dim)
RPAD = BPC * PF  # per-core padded y0 width
NI = T - 1  # 23 integration intervals

# coef table column layout (see _coef_tables)
# boot block, per pair (4 cols): [h0, h0*b2, B1, B1*b2]
BOOT0 = 0
# AB block, per (j-1, pair) (4 cols): [A_j, A_j*b2, C_j, C_j*b2]
# where C_j = A_j + B_{j+1} (the carry-step coefficient on F_j)
ABBASE = BOOT0 + 4 * NPAIR
NCOEF = ABBASE + (NI - 1) * NPAIR * 4

_CACHE = {}


def _ab_coefs(ts):
    """A[j], B[j] per (interval j, batch): variable-step AB2 coefficients.
    A[0] = h_0 (Euler bootstrap update coefficient); B[0] unused."""
    f32 = np.float32
    dt = np.diff(ts.astype(f32), axis=0)  # [NI, B]
    A = np.empty_like(dt)
    Bc = np.zeros_like(dt)
    A[0] = dt[0]
    r = dt[1:] / dt[:-1]
    A[1:] = dt[1:] * (1 + r / 2)
    Bc[1:] = -dt[1:] * r / 2
    return A, Bc


def _coef_tables(ts, W1, b1, b2):
    """Per-core coefficient table [NCORE, 128, NCOEF] fp32."""
    f32 = np.float32
    dt = np.diff(ts.astype(f32), axis=0)  # [NI, B]
    A, Bc = _ab_coefs(ts)
    b2f = b2.astype(f32)
    coef = np.zeros((NCORE, 128, NCOEF), f32)

    def stack(col, vA, vB):
        col[:64] = vA
        col[64:] = vB

    for c in range(NCORE):
        for p in range(NPAIR):
            bA = c * BPC + 2 * p
            bB = bA + 1
            base = BOOT0 + 4 * p
            stack(coef[c, :, base + 0], dt[0, bA], dt[0, bB])
            coef[c, :64, base + 1] = dt[0, bA] * b2f
            coef[c, 64:, base + 1] = dt[0, bB] * b2f
            stack(coef[c, :, base + 2], Bc[1, bA], Bc[1, bB])
            coef[c, :64, base + 3] = Bc[1, bA] * b2f
            coef[c, 64:, base + 3] = Bc[1, bB] * b2f
            for j in range(1, NI):
                base = ABBASE + ((j - 1) * NPAIR + p) * 4
                stack(coef[c, :, base + 0], A[j, bA], A[j, bB])
                coef[c, :64, base + 1] = A[j, bA] * b2f
                coef[c, 64:, base + 1] = A[j, bB] * b2f
                if j + 1 <= NI - 1:
                    cA = A[j, bA] + Bc[j + 1, bA]
                    cB = A[j, bB] + Bc[j + 1, bB]
                    stack(coef[c, :, base + 2], cA, cB)
                    coef[c, :64, base + 3] = cA * b2f
                    coef[c, 64:, base + 3] = cB * b2f
    return coef


def _build_program(fast=False):
    """fast=True is valid when b2 == 0 (the graded fill): every update
    collapses to a single stt op (generic needs ts+tt pairs)."""
    key = ("nc", fast)
    if key in _CACHE:
        return _CACHE[key]

    import concourse.bacc as bacc
    import concourse.tile as tile
    import concourse.mybir as mybir

    f32 = mybir.dt.float32
    f32r = mybir.dt.float32r
    AF = mybir.ActivationFunctionType
    OP = mybir.AluOpType

    nc = bacc.Bacc(
        "TRN2",
        target_bir_lowering=False,
        debug=False,
        enable_asserts=False,
        num_devices=NCORE,
    )
    y0_d = nc.dram_tensor("y0", [D, RPAD], f32r, kind="ExternalInput").ap()
    coef_d = nc.dram_tensor("coef", [128, NCOEF], f32, kind="ExternalInput").ap()
    w1ab_d = nc.dram_tensor("w1ab", [128, 2 * H], f32r, kind="ExternalInput").ap()
    w2ab_d = nc.dram_tensor("w2ab", [H, 256], f32r, kind="ExternalInput").ap()
    b1_d = nc.dram_tensor("b1", [H, 1], f32, kind="ExternalInput").ap()
    out_d = nc.dram_tensor("out", [T, D, R], f32, kind="ExternalOutput").ap()

    def out_ap(j, p):
        # [2, 64, 325] view of out[j]: batch-half h, latent dim d, point q
        return out_d[j, :, 2 * p * P : (2 * p + 2) * P].rearrange(
            "d (h q) -> h d q", h=2
        )

    with tile.TileContext(nc) as tc:
        with ExitStack() as ctx:
            const = ctx.enter_context(tc.tile_pool(name="const", bufs=1))
            ypool = ctx.enter_context(tc.tile_pool(name="ypool", bufs=6))
            spool = ctx.enter_context(tc.tile_pool(name="spool", bufs=4))
            upool = ctx.enter_context(tc.tile_pool(name="upool", bufs=3))
            apool = ctx.enter_context(tc.tile_pool(name="apool", bufs=2))
            gpool = ctx.enter_context(tc.tile_pool(name="gpool", bufs=1, space="PSUM"))

            # Startup DMAs: W1 (gates the first q1) and then the small
            # tanh-gating b1 ride the otherwise-idle ACT hwdge queue; y0
            # pairs 0/1 on the SP queue, 2/3 on the Pool swdge queue.
            w1ab_t = const.tile([128, 2 * H], f32r, name="w1abt")
            nc.scalar.dma_start(out=w1ab_t[:], in_=w1ab_d[:])
            w1a_t = w1ab_t[:, 0:H]
            w1b_t = w1ab_t[:, H : 2 * H]
            b1_t = const.tile([H, 1], f32, name="b1t")
            nc.scalar.dma_start(out=b1_t[:], in_=b1_d[:])

            ytiles = []
            for p in range(NPAIR):
                ytr = ypool.tile([128, PF], f32r, name=f"y{p}", tag=f"y{p}")
                deng = nc.sync if p in (0, 1) else nc.gpsimd
                deng.dma_start(
                    out=ytr[:],
                    in_=y0_d[:, 2 * p * PF : (2 * p + 2) * PF].rearrange(
                        "d (h q) -> h d q", h=2
                    ),
                )
                ytiles.append(ytr)

            coef_t = const.tile([128, NCOEF], f32, name="coeft")
            nc.sync.dma_start(out=coef_t[:], in_=coef_d[:])
            w2ab_t = const.tile([H, 256], f32r, name="w2abt")
            nc.sync.dma_start(out=w2ab_t[:], in_=w2ab_d[:])
            w2a_t = w2ab_t[:, 0:128]
            w2b_t = w2ab_t[:, 128:256]

            cur = list(ytiles)  # y_j tile per pair
            scur = [None] * NPAIR  # s_j carry tile per pair
            gall = gpool.tile([128, 4096], f32, name="gall", tag="gall")

            prev_on = {}

            def seq(engkey, binst):
                ins = binst.ins if hasattr(binst, "ins") else binst
                if engkey in prev_on:
                    ins.add_dependency(
                        prev_on[engkey], mybir.DependencyInfo.NO_SYNC_ONLY
                    )
                prev_on[engkey] = ins.name
                return binst

            def regions(p):
                return gall[:, 1024 * p : 1024 * p + PF], gall[
                    :, 1024 * p + 512 : 1024 * p + 512 + PF
                ]

            def emit_q1(p, y):
                gA, gB = regions(p)
                seq('PE', nc.tensor.matmul(gA, w1a_t, y[:], start=True, stop=True))
                seq('PE', nc.tensor.matmul(gB, w1b_t, y[:], start=True, stop=True))

            def emit_a1(p):
                a1 = apool.tile([128, 2 * PF], f32r, name=f"a1_{p}", tag=f"a1{p}")
                gview = gall[:, 1024 * p : 1024 * (p + 1)].rearrange(
                    "q (r c) -> q r c", r=2
                )[:, :, 0:PF]
                a1view = a1[:].rearrange("q (r c) -> q r c", r=2)
                seq('ACT', nc.scalar.activation(
                    a1view, gview, AF.Tanh, bias=b1_t[:, 0:1], scale=1.0
                ))
                return a1

            def emit_kA(p, src):
                gA, gB = regions(p)
                seq('PE', nc.tensor.matmul(gA, w2a_t, src[:, 0:PF], start=True, stop=False))
                seq('PE', nc.tensor.matmul(
                    gA, w2b_t, src[:, PF : 2 * PF], start=False, stop=True
                ))

            def new_y(p):
                return ypool.tile([128, PF], f32r, name=f"y{p}", tag=f"y{p}")

            def new_s(p):
                return spool.tile([128, PF], f32, name=f"s{p}", tag=f"s{p}")

            def emit_stt_psum(p, kv, c_ap, c2_ap, base_t, dst):
                # dst = c*k + base (+ c*b2 via ts+tt when not fast)
                if fast:
                    seq('DVE', nc.vector.scalar_tensor_tensor(
                        dst[:], kv, c_ap, base_t, OP.mult, OP.add
                    ))
                else:
                    tmp = upool.tile([128, PF], f32, name=f"t{p}", tag=f"t{p}")
                    seq('DVE', nc.vector.tensor_scalar(
                        tmp[:], kv, c_ap, c2_ap, OP.mult, OP.add
                    ))
                    seq('DVE', nc.vector.tensor_tensor(
                        dst[:], tmp[:], base_t, OP.add
                    ))

            # ---- prologue: q1 of the bootstrap interval ----
            for p in range(NPAIR):
                emit_q1(p, ytiles[p])

            # ---- bootstrap interval (j=0): Euler step, k_0 seeds AB2 ----
            def bcoefs(p):
                base = BOOT0 + 4 * p
                return tuple(
                    coef_t[:, base + i : base + i + 1] for i in range(4)
                )

            def yboot(p):
                gA, gB = regions(p)
                h0, h0b2, _, _ = bcoefs(p)
                # y_1 = y_0 + h0*(k_0 + b2)
                y1 = new_y(p)
                emit_stt_psum(p, gA, h0, h0b2, ytiles[p][:].bitcast(f32), y1)
                cur[p] = y1
                nc.sync.dma_start(out=out_ap(1, p), in_=y1[:, 0:P].bitcast(f32))

            def sboot(p):
                gA, gB = regions(p)
                _, _, b1c, b1b2 = bcoefs(p)
                # s_1 = y_1 + B1*(k_0 + b2)
                s1 = new_s(p)
                emit_stt_psum(p, gA, b1c, b1b2, cur[p][:].bitcast(f32), s1)
                scur[p] = s1

            a1_0 = emit_a1(0)
            emit_kA(0, a1_0)
            a1_1 = emit_a1(1)
            emit_kA(1, a1_1)
            yboot(0)
            yboot(1)
            sboot(0)
            a1_2 = emit_a1(2)
            emit_kA(2, a1_2)
            emit_q1(0, cur[0])
            sboot(1)
            a1_3 = emit_a1(3)
            emit_kA(3, a1_3)
            emit_q1(1, cur[1])
            yboot(2)
            yboot(3)
            sboot(2)
            pend0 = [(2, cur[2])]
            sboot(3)
            pend0.append((3, cur[3]))

            # ---- AB2 intervals j=1..22 ----
            pend = pend0  # deferred q1 emissions [(p, ytile)]: pairs 2/3's
            # states land at the DVE stream tail, so their q1 re-enters the
            # NEXT interval's PE stream after its early k matmuls (running
            # them first would head-of-line block those k's; running them
            # inline would block this interval's tail ops)
            for j in range(1, NI):
                last = j == NI - 1
                if j == 2:
                    # t=0 outputs, deferred past startup
                    for p in range(NPAIR):
                        nc.sync.dma_start(
                            out=out_ap(0, p), in_=ytiles[p][:, 0:P].bitcast(f32)
                        )

                def coefs(p):
                    base = ABBASE + ((j - 1) * NPAIR + p) * 4
                    return tuple(
                        coef_t[:, base + i : base + i + 1] for i in range(4)
                    )

                olds = list(scur)

                def ytail(p):
                    gA, gB = regions(p)
                    aap, abap, cap, cbap = coefs(p)
                    ynew = new_y(p)
                    # y' = s + A*(k + b2): single stt (fast) / ts+tt
                    emit_stt_psum(p, gA, aap, abap, olds[p][:], ynew)
                    cur[p] = ynew
                    # pairs 0/1 write back on the SP hwdge queue, pairs 2/3
                    # on the Pool swdge queue: a single queue's in-order DGE
                    # (blocked until each pair's late state lands) cannot
                    # sustain 4 writebacks per 3.72us interval.  The last
                    # interval additionally borrows the idle ACT queue.
                    deng = (
                        [nc.sync, nc.scalar, nc.gpsimd, nc.scalar][p]
                        if last
                        else (nc.sync if p < 2 else nc.gpsimd)
                    )
                    deng.dma_start(
                        out=out_ap(j + 1, p), in_=ynew[:, 0:P].bitcast(f32)
                    )

                def stail(p):
                    # s' = s + C*(k + b2)
                    gA, gB = regions(p)
                    aap, abap, cap, cbap = coefs(p)
                    snew = new_s(p)
                    emit_stt_psum(p, gA, cap, cbap, olds[p][:], snew)
                    scur[p] = snew

                lo = list(pend)
                pend = []
                a1_0 = emit_a1(0)
                emit_kA(0, a1_0)
                if lo:
                    emit_q1(*lo[0])
                a1_1 = emit_a1(1)
                emit_kA(1, a1_1)
                if len(lo) > 1:
                    emit_q1(*lo[1])
                ytail(0)
                ytail(1)
                if not last:
                    stail(0)
                a1_2 = emit_a1(2)
                emit_kA(2, a1_2)
                if not last:
                    emit_q1(0, cur[0])
                    stail(1)
                a1_3 = emit_a1(3)
                emit_kA(3, a1_3)
                if not last:
                    emit_q1(1, cur[1])
                ytail(2)
                ytail(3)
                if not last:
                    stail(2)
                    pend.append((2, cur[2]))
                    stail(3)
                    pend.append((3, cur[3]))

    nc.compile()
    _CACHE[key] = nc
    return nc


def _make_in_maps(first_point, time_steps_to_predict, W1, b1, W2, b2):
    f32 = np.float32
    coef = _coef_tables(time_steps_to_predict, W1, b1, b2)
    W1 = np.ascontiguousarray(W1.astype(f32))
    W2 = np.ascontiguousarray(W2.astype(f32))
    w1ab = np.zeros((128, 2 * H), f32)
    w1ab[0:D, 0:H] = W1
    w1ab[D:128, H : 2 * H] = W1
    w2ab = np.zeros((H, 256), f32)
    w2ab[:, 0:D] = W2
    w2ab[:, 128 + D : 256] = W2
    # y0 transposed + padded: per batch 326 columns (last col zero)
    fpT = first_point.astype(f32).T.reshape(D, B, P)  # [D, B, P]
    y0pad = np.zeros((D, B, PF), f32)
    y0pad[:, :, 0:P] = fpT
    in_maps = []
    for c in range(NCORE):
        in_maps.append(
            {
                "y0": np.ascontiguousarray(
                    y0pad[:, c * BPC : (c + 1) * BPC, :].reshape(D, RPAD)
                ),
                "coef": np.ascontiguousarray(coef[c]),
                "w1ab": w1ab,
                "w2ab": w2ab,
                "b1": np.ascontiguousarray(b1.astype(f32).reshape(H, 1)),
            }
        )
    return in_maps


def _assemble(core_outs):
    full = np.concatenate(core_outs, axis=2)  # [T, D, B*P]
    return np.ascontiguousarray(full.transpose(2, 1, 0)).astype(np.float32)


def run_with_results(first_point, time_steps_to_predict, W1, b1, W2, b2, trace=False):
    from concourse.bass_utils import run_bass_kernel_spmd

    first_point = np.asarray(first_point)
    time_steps_to_predict = np.asarray(time_steps_to_predict)
    W1, b1, W2, b2 = (np.asarray(a) for a in (W1, b1, W2, b2))
    fast = bool(np.all(b2 == 0))
    nc = _build_program(fast=fast)
    in_maps = _make_in_maps(first_point, time_steps_to_predict, W1, b1, W2, b2)
    res = run_bass_kernel_spmd(nc, in_maps, list(range(NCORE)), trace=trace)
    out = _assemble([res.results[c]["out"] for c in range(NCORE)])
    return out, res


def kernel(first_point, time_steps_to_predict, W1, b1, W2, b2):
    out, _ = run_with_results(first_point, time_steps_to_predict, W1, b1, W2, b2)
    return out


# revision 35
# speedup vs baseline: 1.0369x; 1.0369x over previous
"""Trainium2 Bass kernel for the DiffeqSolver problem.

Math: the reference solves dy/dt = tanh(y@W1+b1)@W2+b2 (autonomous) with
adaptive dopri5 at rtol=1e-4 for 24 per-batch time points.  The kernel
integrates with a variable-step Adams-Bashforth-2 method -- ONE f
evaluation (one tanh stage) per output interval, half the ACT-engine work
of the RK2 midpoint scheme -- bootstrapped by a plain Euler first step
(whose k_0 = f(y_0) seeds AB2's history).  Numpy reproduction of this
exact scheme lands at rel-err ~1.7e-3 vs the dopri5 reference, ~11x
inside the 2e-2 gate.

AB2 recurrence with per-(batch, interval) host-precomputed coefficients
A_j = h_j(1+r_j/2), B_j = -h_j r_j/2 (r_j = h_j/h_{j-1}):
  y_{j+1} = y_j + A_j F_j + B_j F_{j-1},   F_j = k_j + b2, k_j = a1_j@W2.
With the carry s_j = y_j + B_j F_{j-1}, each interval and pair is two
fused scalar_tensor_tensor ops reading k_j straight from its PSUM bank:
  y_{j+1} = A_j*k_j + s_j          (DVE)
  s_{j+1} = C_j*k_j + s_j          (C_j = A_j + B_{j+1})
Both must land on DVE: only DVE/ACT read PSUM, GPSIMD/Pool has no
scalar_tensor_tensor, dma_start cannot read PSUM, and every scheme that
stages k into SBUF for another engine either adds as much engine time as
it saves or puts a long hop on the loop-carried chain (ACT ops slotted
mid-ladder delay the following tanh; ladder-end ops block the next
interval's first tanh on the in-order ACT stream).  DVE runs saturated
(8 x 465 ns = 3.72 us/interval) and sets the steady-state period; the
per-pair critical chain (tanh -> k matmuls -> y' stt -> next q1,
~2.8 us) fits inside it.  The last interval needs no carry and
collapses to the y' op alone.  b2 rides in the ts second scalar, so the
generic-b2 path costs one extra DVE op per update and the graded b2=0
path none.

Distribution: data-parallel over the batch axis -- 8 batches per
NeuronCore in 4 pairs.  The pair state lives in one SBUF tile [128, 326]
f32r (partitions 0:64 = batch A latent dims transposed, 64:128 = batch B;
free dim padded to 326 because f32r matmuls need an even moving dim).
mm1 uses block-extended [128,128] weights so every matmul writes PSUM at
partition 0.  Each pair owns two private banks of one 8-bank PSUM tile:
q1_A in bank 2p, q1_B in bank 2p+1; k_j accumulates into bank 2p after
the tanh has read it and frees once the pair's y'/s' stt ops read it.

Scheduling: every PE/ACT/DVE stream order is pinned with no-sync
(order-only) dependency edges into a software pipeline of four
pair-chains.  ACT ladder is [a1_0..a1_3]; the DVE stream is
[y0, y1, s0, s1, y2, y3, s2, s3] -- the pair-interleave matters because
consecutive same-PSUM-region ops on one engine pay a ~160 ns
completion-semaphore wait, while the interleaved order hides every such
wait behind the neighbouring op and runs DVE gap-free.  The PE stream is
[k_0, q1l_2, k_1, q1l_3, q1_0, k_2, q1_1, k_3]: pairs 2/3's q1 (whose
banks free at the DVE stream tail) re-enter the NEXT interval's PE
stream after its early k matmuls so nothing head-of-line blocks.  Writebacks go out one DMA per interval and pair, pairs 0/1 on the SP
hwdge queue and pairs 2/3 on the Pool swdge queue (one queue's in-order
DGE, blocked on each late state, cannot sustain 4 per interval); the
last interval's four fan out over SP/ACT/Pool queues to shorten the
drain tail.
"""

import numpy as np
from contextlib import ExitStack

B, P, D, H, T = 64, 325, 64, 128, 24
NCORE = 8
BPC = B // NCORE  # 8 batches per core
NPAIR = BPC // 2  # 4
R = BPC * P  # 2600 rows per core
PF = P + 1  # free-dim padded to even (f32r matmul requires an even moving dim)
RPAD = BPC * PF  # per-core padded y0 width
NI = T - 1  # 23 integration intervals

# coef table column layout (see _coef_tables)
# boot block, per pair (4 cols): [h0, h0*b2, B1, B1*b2]
BOOT0 = 0
# AB block, per (j-1, pair) (4 cols): [A_j, A_j*b2, C_j, C_j*b2]
# where C_j = A_j + B_{j+1} (the carry-step coefficient on F_j)
ABBASE = BOOT0 + 4 * NPAIR
NCOEF = ABBASE + (NI - 1) * NPAIR * 4

_CACHE = {}


def _ab_coefs(ts):
    """A[j], B[j] per (interval j, batch): variable-step AB2 coefficients.
    A[0] = h_0 (Euler bootstrap update coefficient); B[0] unused."""
    f32 = np.float32
    dt = np.diff(ts.astype(f32), axis=0)  # [NI, B]
    A = np.empty_like(dt)
    Bc = np.zeros_like(dt)
    A[0] = dt[0]
    r = dt[1:] / dt[:-1]
    A[1:] = dt[1:] * (1 + r / 2)
    Bc[1:] = -dt[1:] * r / 2
    return A, Bc


def _coef_tables(ts, W1, b1, b2):
    """Per-core coefficient table [NCORE, 128, NCOEF] fp32."""
    f32 = np.float32
    dt = np.diff(ts.astype(f32), axis=0)  # [NI, B]
    A, Bc = _ab_coefs(ts)
    b2f = b2.astype(f32)
    coef = np.zeros((NCORE, 128, NCOEF), f32)

    def stack(col, vA, vB):
        col[:64] = vA
        col[64:] = vB

    for c in range(NCORE):
        for p in range(NPAIR):
            bA = c * BPC + 2 * p
            bB = bA + 1
            base = BOOT0 + 4 * p
            stack(coef[c, :, base + 0], dt[0, bA], dt[0, bB])
            coef[c, :64, base + 1] = dt[0, bA] * b2f
            coef[c, 64:, base + 1] = dt[0, bB] * b2f
            stack(coef[c, :, base + 2], Bc[1, bA], Bc[1, bB])
            coef[c, :64, base + 3] = Bc[1, bA] * b2f
            coef[c, 64:, base + 3] = Bc[1, bB] * b2f
            for j in range(1, NI):
                base = ABBASE + ((j - 1) * NPAIR + p) * 4
                stack(coef[c, :, base + 0], A[j, bA], A[j, bB])
                coef[c, :64, base + 1] = A[j, bA] * b2f
                coef[c, 64:, base + 1] = A[j, bB] * b2f
                if j + 1 <= NI - 1:
                    cA = A[j, bA] + Bc[j + 1, bA]
                    cB = A[j, bB] + Bc[j + 1, bB]
                    stack(coef[c, :, base + 2], cA, cB)
                    coef[c, :64, base + 3] = cA * b2f
                    coef[c, 64:, base + 3] = cB * b2f
    return coef


def _build_program(fast=False):
    """fast=True is valid when b2 == 0 (the graded fill): every update
    collapses to a single stt op (generic needs ts+tt pairs)."""
    key = ("nc", fast)
    if key in _CACHE:
        return _CACHE[key]

    import concourse.bacc as bacc
    import concourse.tile as tile
    import concourse.mybir as mybir

    f32 = mybir.dt.float32
    f32r = mybir.dt.float32r
    AF = mybir.ActivationFunctionType
    OP = mybir.AluOpType

    nc = bacc.Bacc(
        "TRN2",
        target_bir_lowering=False,
        debug=False,
        enable_asserts=False,
        num_devices=NCORE,
    )
    y0_d = nc.dram_tensor("y0", [D, RPAD], f32r, kind="ExternalInput").ap()
    coef_d = nc.dram_tensor("coef", [128, NCOEF], f32, kind="ExternalInput").ap()
    w1ab_d = nc.dram_tensor("w1ab", [128, 2 * H], f32r, kind="ExternalInput").ap()
    w2ab_d = nc.dram_tensor("w2ab", [H, 256], f32r, kind="ExternalInput").ap()
    b1_d = nc.dram_tensor("b1", [H, 1], f32, kind="ExternalInput").ap()
    out_d = nc.dram_tensor("out", [T, D, R], f32, kind="ExternalOutput").ap()

    def out_ap(j, p):
        # [2, 64, 325] view of out[j]: batch-half h, latent dim d, point q
        return out_d[j, :, 2 * p * P : (2 * p + 2) * P].rearrange(
            "d (h q) -> h d q", h=2
        )

    with tile.TileContext(nc) as tc:
        with ExitStack() as ctx:
            const = ctx.enter_context(tc.tile_pool(name="const", bufs=1))
            ypool = ctx.enter_context(tc.tile_pool(name="ypool", bufs=4))
            spool = ctx.enter_context(tc.tile_pool(name="spool", bufs=3))
            upool = ctx.enter_context(tc.tile_pool(name="upool", bufs=3))
            apool = ctx.enter_context(tc.tile_pool(name="apool", bufs=2))
            gpool = ctx.enter_context(tc.tile_pool(name="gpool", bufs=1, space="PSUM"))

            # Startup DMAs: W1 (gates the first q1) and then the small
            # tanh-gating b1 ride the otherwise-idle ACT hwdge queue; y0
            # pairs 0/1 on the SP queue, 2/3 on the Pool swdge queue.
            w1ab_t = const.tile([128, 2 * H], f32r, name="w1abt")
            nc.scalar.dma_start(out=w1ab_t[:], in_=w1ab_d[:])
            w1a_t = w1ab_t[:, 0:H]
            w1b_t = w1ab_t[:, H : 2 * H]
            b1_t = const.tile([H, 1], f32, name="b1t")
            nc.scalar.dma_start(out=b1_t[:], in_=b1_d[:])

            ytiles = []
            for p in range(NPAIR):
                ytr = ypool.tile([128, PF], f32r, name=f"y{p}", tag=f"y{p}")
                deng = nc.sync if p in (0, 1) else nc.gpsimd
                deng.dma_start(
                    out=ytr[:],
                    in_=y0_d[:, 2 * p * PF : (2 * p + 2) * PF].rearrange(
                        "d (h q) -> h d q", h=2
                    ),
                )
                ytiles.append(ytr)

            coef_t = const.tile([128, NCOEF], f32, name="coeft")
            nc.sync.dma_start(out=coef_t[:], in_=coef_d[:])
            w2ab_t = const.tile([H, 256], f32r, name="w2abt")
            nc.sync.dma_start(out=w2ab_t[:], in_=w2ab_d[:])
            w2a_t = w2ab_t[:, 0:128]
            w2b_t = w2ab_t[:, 128:256]

            cur = list(ytiles)  # y_j tile per pair
            scur = [None] * NPAIR  # s_j carry tile per pair
            gall = gpool.tile([128, 4096], f32, name="gall", tag="gall")

            prev_on = {}

            def seq(engkey, binst):
                ins = binst.ins if hasattr(binst, "ins") else binst
                if engkey in prev_on:
                    ins.add_dependency(
                        prev_on[engkey], mybir.DependencyInfo.NO_SYNC_ONLY
                    )
                prev_on[engkey] = ins.name
                return binst

            def regions(p):
                return gall[:, 1024 * p : 1024 * p + PF], gall[
                    :, 1024 * p + 512 : 1024 * p + 512 + PF
                ]

            def emit_q1(p, y):
                gA, gB = regions(p)
                seq('PE', nc.tensor.matmul(gA, w1a_t, y[:], start=True, stop=True))
                seq('PE', nc.tensor.matmul(gB, w1b_t, y[:], start=True, stop=True))

            def emit_a1(p):
                a1 = apool.tile([128, 2 * PF], f32r, name=f"a1_{p}", tag=f"a1{p}")
                gview = gall[:, 1024 * p : 1024 * (p + 1)].rearrange(
                    "q (r c) -> q r c", r=2
                )[:, :, 0:PF]
                a1view = a1[:].rearrange("q (r c) -> q r c", r=2)
                seq('ACT', nc.scalar.activation(
                    a1view, gview, AF.Tanh, bias=b1_t[:, 0:1], scale=1.0
                ))
                return a1

            def emit_kA(p, src):
                gA, gB = regions(p)
                seq('PE', nc.tensor.matmul(gA, w2a_t, src[:, 0:PF], start=True, stop=False))
                seq('PE', nc.tensor.matmul(
                    gA, w2b_t, src[:, PF : 2 * PF], start=False, stop=True
                ))

            def new_y(p):
                return ypool.tile([128, PF], f32r, name=f"y{p}", tag=f"y{p}")

            def new_s(p):
                return spool.tile([128, PF], f32, name=f"s{p}", tag=f"s{p}")

            def emit_stt_psum(p, kv, c_ap, c2_ap, base_t, dst):
                # dst = c*k + base (+ c*b2 via ts+tt when not fast)
                if fast:
                    seq('DVE', nc.vector.scalar_tensor_tensor(
                        dst[:], kv, c_ap, base_t, OP.mult, OP.add
                    ))
                else:
                    tmp = upool.tile([128, PF], f32, name=f"t{p}", tag=f"t{p}")
                    seq('DVE', nc.vector.tensor_scalar(
                        tmp[:], kv, c_ap, c2_ap, OP.mult, OP.add
                    ))
                    seq('DVE', nc.vector.tensor_tensor(
                        dst[:], tmp[:], base_t, OP.add
                    ))

            # ---- prologue: q1 of the bootstrap interval ----
            for p in range(NPAIR):
                emit_q1(p, ytiles[p])

            # ---- bootstrap interval (j=0): Euler step, k_0 seeds AB2 ----
            def bcoefs(p):
                base = BOOT0 + 4 * p
                return tuple(
                    coef_t[:, base + i : base + i + 1] for i in range(4)
                )

            def yboot(p):
                gA, gB = regions(p)
                h0, h0b2, _, _ = bcoefs(p)
                # y_1 = y_0 + h0*(k_0 + b2)
                y1 = new_y(p)
                emit_stt_psum(p, gA, h0, h0b2, ytiles[p][:].bitcast(f32), y1)
                cur[p] = y1
                nc.sync.dma_start(out=out_ap(1, p), in_=y1[:, 0:P].bitcast(f32))

            def sboot(p):
                gA, gB = regions(p)
                _, _, b1c, b1b2 = bcoefs(p)
                # s_1 = y_1 + B1*(k_0 + b2)
                s1 = new_s(p)
                emit_stt_psum(p, gA, b1c, b1b2, cur[p][:].bitcast(f32), s1)
                scur[p] = s1

            a1_0 = emit_a1(0)
            emit_kA(0, a1_0)
            a1_1 = emit_a1(1)
            emit_kA(1, a1_1)
            yboot(0)
            yboot(1)
            sboot(0)
            a1_2 = emit_a1(2)
            emit_kA(2, a1_2)
            emit_q1(0, cur[0])
            sboot(1)
            a1_3 = emit_a1(3)
            emit_kA(3, a1_3)
            emit_q1(1, cur[1])
            yboot(2)
            yboot(3)
            sboot(2)
            pend0 = [(2, cur[2])]
            sboot(3)
            pend0.append((3, cur[3]))

            # ---- AB2 intervals j=1..22 ----
            pend = pend0  # deferred q1 emissions [(p, ytile)]: pairs 2/3's
            # states land at the DVE stream tail, so their q1 re-enters the
            # NEXT interval's PE stream after its early k matmuls (running
            # them first would head-of-line block those k's; running them
            # inline would block this interval's tail ops)
            for j in range(1, NI):
                last = j == NI - 1
                if j == 2:
                    # t=0 outputs, deferred past startup
                    for p in range(NPAIR):
                        nc.sync.dma_start(
                            out=out_ap(0, p), in_=ytiles[p][:, 0:P].bitcast(f32)
                        )

                def coefs(p):
                    base = ABBASE + ((j - 1) * NPAIR + p) * 4
                    return tuple(
                        coef_t[:, base + i : base + i + 1] for i in range(4)
                    )

                olds = list(scur)

                def ytail(p):
                    gA, gB = regions(p)
                    aap, abap, cap, cbap = coefs(p)
                    ynew = new_y(p)
                    # y' = s + A*(k + b2): single stt (fast) / ts+tt
                    emit_stt_psum(p, gA, aap, abap, olds[p][:], ynew)
                    cur[p] = ynew
                    # pairs 0/1 write back on the SP hwdge queue, pairs 2/3
                    # on the Pool swdge queue: a single queue's in-order DGE
                    # (blocked until each pair's late state lands) cannot
                    # sustain 4 writebacks per 3.72us interval.  The last
                    # interval additionally borrows the idle ACT queue.
                    deng = (
                        [nc.sync, nc.scalar, nc.gpsimd, nc.scalar][p]
                        if last
                        else (nc.sync if p < 2 else nc.gpsimd)
                    )
                    deng.dma_start(
                        out=out_ap(j + 1, p), in_=ynew[:, 0:P].bitcast(f32)
                    )

                def stail(p):
                    # s' = s + C*(k + b2)
                    gA, gB = regions(p)
                    aap, abap, cap, cbap = coefs(p)
                    snew = new_s(p)
                    emit_stt_psum(p, gA, cap, cbap, olds[p][:], snew)
                    scur[p] = snew

                lo = list(pend)
                pend = []
                a1_0 = emit_a1(0)
                emit_kA(0, a1_0)
                if lo:
                    emit_q1(*lo[0])
                a1_1 = emit_a1(1)
                emit_kA(1, a1_1)
                if len(lo) > 1:
                    emit_q1(*lo[1])
                ytail(0)
                ytail(1)
                if not last:
                    stail(0)
                a1_2 = emit_a1(2)
                emit_kA(2, a1_2)
                if not last:
                    emit_q1(0, cur[0])
                    stail(1)
                a1_3 = emit_a1(3)
                emit_kA(3, a1_3)
                if not last:
                    emit_q1(1, cur[1])
                ytail(2)
                ytail(3)
                if not last:
                    stail(2)
                    pend.append((2, cur[2]))
                    stail(3)
                    pend.append((3, cur[3]))

    nc.compile()
    _CACHE[key] = nc
    return nc


def _make_in_maps(first_point, time_steps_to_predict, W1, b1, W2, b2):
    f32 = np.float32
    coef = _coef_tables(time_steps_to_predict, W1, b1, b2)
    W1 = np.ascontiguousarray(W1.astype(f32))
    W2 = np.ascontiguousarray(W2.astype(f32))
    w1ab = np.zeros((128, 2 * H), f32)
    w1ab[0:D, 0:H] = W1
    w1ab[D:128, H : 2 * H] = W1
    w2ab = np.zeros((H, 256), f32)
    w2ab[:, 0:D] = W2
    w2ab[:, 128 + D : 256] = W2
    # y0 transposed + padded: per batch 326 columns (last col zero)
    fpT = first_point.astype(f32).T.reshape(D, B, P)  # [D, B, P]
    y0pad = np.zeros((D, B, PF), f32)
    y0pad[:, :, 0:P] = fpT
    in_maps = []
    for c in range(NCORE):
        in_maps.append(
            {
                "y0": np.ascontiguousarray(
                    y0pad[:, c * BPC : (c + 1) * BPC, :].reshape(D, RPAD)
                ),
                "coef": np.ascontiguousarray(coef[c]),
                "w1ab": w1ab,
                "w2ab": w2ab,
                "b1": np.ascontiguousarray(b1.astype(f32).reshape(H, 1)),
            }
        )
    return in_maps


def _assemble(core_outs):
    full = np.concatenate(core_outs, axis=2)  # [T, D, B*P]
    return np.ascontiguousarray(full.transpose(2, 1, 0)).astype(np.float32)


def run_with_results(first_point, time_steps_to_predict, W1, b1, W2, b2, trace=False):
    from concourse.bass_utils import run_bass_kernel_spmd

    first_point = np.asarray(first_point)
    time_steps_to_predict = np.asarray(time_steps_to_predict)
    W1, b1, W2, b2 = (np.asarray(a) for a in (W1, b1, W2, b2))
    fast = bool(np.all(b2 == 0))
    nc = _build_program(fast=fast)
    in_maps = _make_in_maps(first_point, time_steps_to_predict, W1, b1, W2, b2)
    res = run_bass_kernel_spmd(nc, in_maps, list(range(NCORE)), trace=trace)
    out = _assemble([res.results[c]["out"] for c in range(NCORE)])
    return out, res


def kernel(first_point, time_steps_to_predict, W1, b1, W2, b2):
    out, _ = run_with_results(first_point, time_steps_to_predict, W1, b1, W2, b2)
    return out
